# revision 64
# baseline (speedup 1.0000x reference)
"""Trainium2 Bass kernel for causal GQA multi-head attention (nn_MHA_79362405695575).

Full (unsharded) inputs -> full output. Internally: tensor-parallel over heads
across 8 NeuronCores. Core c owns q-heads [4c,4c+4) and kv-head c. After
attention, a small bf16 AllToAll (chunked x4, overlapped with attention)
converts head-sharding to row-sharding; each core then runs the full
out-projection for its own 512 rows of (B*S) and returns y^T for those rows.

Reference semantics (fp32):
  q = x@Wq; k = x@Wk; v = x@Wv + bv           (B=2, S=2048, D=2048)
  q,k := interleaved RoPE(base 10000, hd=64)
  scores = q k^T / 8 (causal), attn = softmax
  out = attn @ v;  y = out @ Wo + bo

All matmul operands are bf16 (PSUM accumulation f32; ~3e-3 rel err, full PE
rate). Everything on-chip is transposed: qT/kT/vT [dim, row] layouts so no PE
transposes are needed anywhere in attention. Softmax is max-free (scores are
provably small) and denominators ride along the AV matmul as a 65th column
of v. Projections (stage 1) are interleaved into the attention stream so the
scalar-engine exp latency of short early spans hides under projection matmuls.
"""

import numpy as np
import ml_dtypes

import concourse.bass as bass
import concourse.tile as tile
from concourse import bacc, mybir
from concourse.bass_utils import run_bass_kernel_spmd

# ---- problem constants (hardcoded; kernel.py must be self-contained) ----
B, S, D = 2, 2048, 2048
NH, NKV, HD = 32, 8, 64
ROPE_BASE = 10000.0
NC = 8                    # cores
HPC = NH // NC            # q heads per core = 4
R = B * S                 # 4096 rows
RS_N = 8                  # projection row spans
RS_W = R // RS_N          # 512 rows per span
QS_W = 512                # attention q-span width
QS_N = 4                  # q spans per batch
KB_W = 128                # k block width
NKB = S // KB_W           # 16 k blocks per batch
NCHK = 4                  # all-to-all chunks (2 spans each)
CRW = R // NCHK // NC     # rows per core per chunk = 128

F32 = mybir.dt.float32
BF = mybir.dt.bfloat16
BF_NP = ml_dtypes.bfloat16

_CACHE = {}

# phase attribution for trace analysis: (tag, #instructions-emitted-so-far)
# pairs + the emission-ordered instruction names. Pure-python bookkeeping —
# the emitted BIR (and thus the NEFF cache key) is unchanged.
PHASE_MARKS = []
NAME_ORDER = None


def _build():
    global NAME_ORDER
    PHASE_MARKS.clear()
    nc = bacc.Bacc("TRN2", target_bir_lowering=False, debug=False, num_devices=NC)

    def _mark(tag):
        PHASE_MARKS.append((tag, len(nc._state.inst_map)))

    # ---- DRAM I/O (pre-tiled on host) ----
    xta = nc.dram_tensor("xta", [RS_N, 128, 8, RS_W], BF, kind="ExternalInput").ap()
    xtb = nc.dram_tensor("xtb", [RS_N, 128, 8, RS_W], BF, kind="ExternalInput").ap()
    wq = nc.dram_tensor("wq", [128, 2, D // 128, 128], BF, kind="ExternalInput").ap()
    wkv = nc.dram_tensor("wkv", [128, D // 128, 128], BF, kind="ExternalInput").ap()
    wo = nc.dram_tensor("wo", [128, D // 128, D], BF, kind="ExternalInput").ap()
    bv_in = nc.dram_tensor("bv", [HD, 1], F32, kind="ExternalInput").ap()
    c4h = nc.dram_tensor("c4h", [128, S], BF, kind="ExternalInput").ap()
    s4h = nc.dram_tensor("s4h", [128, S], BF, kind="ExternalInput").ap()
    p2 = nc.dram_tensor("p2", [128, 128], BF, kind="ExternalInput").ap()
    ident = nc.dram_tensor("ident", [64, 64], F32, kind="ExternalInput").ap()
    zm = nc.dram_tensor("zm", [128, 128], BF, kind="ExternalInput").ap()
    y_sh = nc.dram_tensor("y_sh", [NCHK * CRW, D], F32, kind="ExternalOutput").ap()

    DMA = nc.sync

    with tile.TileContext(nc) as tc:
        with (
            tc.tile_pool(name="persist", bufs=1) as pp,
            tc.tile_pool(name="dram", bufs=1, space="DRAM") as dram,
        ):
            # ---- persistent SBUF (whole kernel) ----
            qrT = [pp.tile([128, R], BF, tag=f"qrT{t}", name=f"qrT{t}") for t in range(2)]
            krT = pp.tile([128, R], BF, tag="krT")
            v_aug = pp.tile([128, R // KB_W, 65], BF, tag="vaug")
            wo_sb = pp.tile([128, D // 128, D], BF, tag="wo")
            p2_sb = pp.tile([128, 128], BF, tag="p2")
            id_sb = pp.tile([64, 64], F32, tag="ident")
            bv_sb = pp.tile([HD, 1], F32, tag="bv")
            zm_sb = pp.tile([128, 128], BF, tag="zm")

            DMA.dma_start(out=p2_sb[:], in_=p2[:])
            DMA.dma_start(out=id_sb[:], in_=ident[:])
            DMA.dma_start(out=bv_sb[:], in_=bv_in[:])
            DMA.dma_start(out=zm_sb[:], in_=zm[:])
            # contiguous whole-tile memset (a strided [:, :, 64:65] memset
            # costs ~14us on the DVE); v copies later overwrite cols 0:64,
            # leaving the ones in col 64
            nc.vector.memset(v_aug[:, :, :], 1.0)

            a2a_in = [dram.tile([16, 128, CRW], BF, tag=f"a2ai{k}", name=f"a2ai{k}")
                      for k in range(NCHK)]
            a2a_out = [dram.tile([16, 128, CRW], BF, tag=f"a2ao{k}", name=f"a2ao{k}")
                       for k in range(NCHK)]
            # last chunk ships per-g so its first collective overlaps the
            # final span's attention (shrinks the end-of-kernel tail)
            a2a_in3 = [dram.tile([8, 128, CRW], BF, tag=f"a2ai3{g}",
                                 name=f"a2ai3{g}") for g in range(2)]
            a2a_out3 = [dram.tile([8, 128, CRW], BF, tag=f"a2ao3{g}",
                                  name=f"a2ao3{g}") for g in range(2)]

            # warmup collective: absorbs the first-collective entry barrier
            # (which eats ALL core-start skew) + firmware setup during stage
            # 1. Tiny payload staged from p2_sb (first DMA, lands ~1us) so
            # the gpsimd queue enters the barrier immediately — the barrier
            # blocks the gpsimd queue (normalize broadcasts + a2a staging),
            # so entering late delays every chunk downstream.
            wu_in = dram.tile([16, 128, 4], BF, tag="wu_i", name="wu_i")
            wu_out = dram.tile([16, 128, 4], BF, tag="wu_o", name="wu_o")
            nc.gpsimd.dma_start(
                out=wu_in.rearrange("b p w -> p b w"),
                in_=p2_sb[:, 0:64].rearrange("p (b w) -> p b w", w=4))
            nc.gpsimd.collective_compute(
                "AllToAll", mybir.AluOpType.bypass,
                replica_groups=[list(range(NC))],
                ins=[wu_in[:]], outs=[wu_out[:]],
            )

            with (
                tc.tile_pool(name="ptp", bufs=3) as ptp,
                tc.tile_pool(name="normp", bufs=2) as normp,
                tc.tile_pool(name="denp", bufs=1) as denp,
                tc.tile_pool(name="sop", bufs=2) as sop,
                tc.tile_pool(name="avp", bufs=2) as avp,
                tc.tile_pool(name="ystg", bufs=5) as ystg,
                tc.tile_pool(name="ps_s", bufs=2, space="PSUM") as ps_s,
                tc.tile_pool(name="ps_av", bufs=1, space="PSUM") as ps_av,
            ):
                def attn_span(s, pump=None):
                    k, sp = divmod(s, 2)
                    b, qs = divmod(s, QS_N)
                    n_kb = 4 * (qs + 1)
                    qsl = slice(b * S + qs * QS_W, b * S + (qs + 1) * QS_W)
                    for g in range(2):
                        pav = ps_av.tile([65, 2 * QS_W], F32, tag="pav")
                        pts = {}

                        def emit_scores(kb):
                            # scores pair (concurrent on disjoint PE row
                            # groups) + exp + diagonal mask for one k-block
                            kbl = slice(b * S + kb * KB_W,
                                        b * S + (kb + 1) * KB_W)
                            off = max(kb - 4 * qs, 0) * 128
                            pss = ps_s.tile([128, 2 * QS_W], F32, tag="pss")
                            for u in range(2):
                                # both u trimmed to off; the exp below reads
                                # the [QS_W, QS_W+off) gap as stale-but-
                                # finite PSUM; its output there is never
                                # consumed
                                usl = slice(u * 64, (u + 1) * 64)
                                nc.tensor.matmul(
                                    pss[:, u * QS_W + off:(u + 1) * QS_W],
                                    krT[usl, kbl],
                                    qrT[g][usl, qsl.start + off:qsl.stop],
                                    start=True, stop=True)
                            pt = ptp.tile([128, 2 * QS_W], BF, tag="pt")
                            pts[kb] = pt
                            nc.scalar.activation(
                                out=pt[:, off:2 * QS_W],
                                in_=pss[:, off:2 * QS_W],
                                func=mybir.ActivationFunctionType.Exp,
                                scale=float(HD) ** -0.5)
                            if kb - 4 * qs >= 0:
                                # triangle mask on the 128 diagonal cols of
                                # each head's valid range
                                for u in range(2):
                                    nc.vector.tensor_tensor(
                                        out=pt[:, u * QS_W + off:
                                            u * QS_W + off + 128],
                                        in0=pt[:, u * QS_W + off:
                                            u * QS_W + off + 128],
                                        in1=zm_sb[:],
                                        op=mybir.AluOpType.mult)

                        # software-pipelined by one k-block: scores(kb+1) and
                        # pump work sit between exp(kb) and AV(kb) in the
                        # in-order PE queue, hiding the ACT exp latency
                        emit_scores(0)
                        for kb in range(n_kb):
                            _mark(f"at{s}g{g}")
                            if kb + 1 < n_kb:
                                emit_scores(kb + 1)
                            if pump is not None:
                                pump(s, g, kb)
                            _mark(f"at{s}g{g}")
                            off = max(kb - 4 * qs, 0) * 128
                            pt = pts.pop(kb)
                            for u in range(2):
                                nc.tensor.matmul(
                                    pav[:, u * QS_W + off:(u + 1) * QS_W],
                                    v_aug[:, b * NKB + kb, :],
                                    pt[:, u * QS_W + off:(u + 1) * QS_W],
                                    start=(kb == 0),
                                    stop=(kb == n_kb - 1),
                                    skip_group_check=True)
                        # normalize heads 2g, 2g+1 and stage for AllToAll.
                        # reciprocal_approx_fast (~18 bits, plenty vs bf16;
                        # ~5x faster than reciprocal()). NOTE: the custom-DVE
                        # op misreads PSUM inputs on hw, so it must read the
                        # SBUF copy, not pav directly.
                        _mark(f"nm{s}g{g}")
                        # normalize: the custom-DVE reciprocal_approx_fast
                        # misreads inputs at non-zero base partition, so first
                        # hop the denominator row (pav row 64) to partition 0
                        # with a plain ACT copy (64->0 remap is 32-aligned,
                        # legal for standard ops), then invert on the DVE.
                        den0 = denp.tile([1, 2 * QS_W], F32, tag="den0")
                        nc.vector.tensor_copy(out=den0[:], in_=pav[64:65, :])
                        den = denp.tile([1, 2 * QS_W], F32, tag="den")
                        nc.vector.reciprocal_approx_fast(
                            out=den[:], in_=den0[:])
                        pavs = normp.tile([65, 2 * QS_W], F32, tag="pavs")
                        nc.scalar.copy(out=pavs[:], in_=pav[:])
                        rb = normp.tile([64, 2 * QS_W], F32, tag="rb")
                        nc.gpsimd.partition_broadcast(rb[:], den[:])
                        so = sop.tile([128, QS_W], BF, tag="so")
                        for u in range(2):
                            nc.vector.tensor_tensor(
                                out=so[u * 64:(u + 1) * 64, :],
                                in0=pavs[0:64, u * QS_W:(u + 1) * QS_W],
                                in1=rb[:, u * QS_W:(u + 1) * QS_W],
                                op=mybir.AluOpType.mult)
                        # scatter: block 2j+g of a2a_in[k] = my rows for core j
                        if k < NCHK - 1:
                            nc.gpsimd.dma_start(
                                out=a2a_in[k][8 * sp + g: 8 * sp + 8: 2]
                                .rearrange("j p w -> p j w"),
                                in_=so.rearrange("p (j w) -> p j w", w=CRW))
                        else:
                            nc.gpsimd.dma_start(
                                out=a2a_in3[g][4 * sp: 4 * sp + 4]
                                .rearrange("j p w -> p j w"),
                                in_=so.rearrange("p (j w) -> p j w", w=CRW))
                            if sp == 1:
                                nc.gpsimd.collective_compute(
                                    "AllToAll", mybir.AluOpType.bypass,
                                    replica_groups=[list(range(NC))],
                                    ins=[a2a_in3[g][:]],
                                    outs=[a2a_out3[g][:]],
                                )

                def emit_cc(k):
                    nc.gpsimd.collective_compute(
                        "AllToAll", mybir.AluOpType.bypass,
                        replica_groups=[list(range(NC))],
                        ins=[a2a_in[k][:]], outs=[a2a_out[k][:]],
                    )

                # ---- stage 1 (projections + RoPE), interleaved below ----
                with (
                    tc.tile_pool(name="w1p", bufs=1) as w1p,
                    tc.tile_pool(name="xtpa", bufs=2) as xtpa,
                    tc.tile_pool(name="xtpb", bufs=2) as xtpb,
                    tc.tile_pool(name="ropet", bufs=2) as ropet,
                    tc.tile_pool(name="vstg", bufs=1) as vstg,
                    tc.tile_pool(name="ps1", bufs=2, space="PSUM") as ps1,
                ):
                    wq_sb = w1p.tile([128, 2, D // 128, 128], BF, tag="wq")
                    wkv_sb = w1p.tile([128, D // 128, 128], BF, tag="wkv")
                    c4_sb = w1p.tile([128, S], BF, tag="c4")
                    s4_sb = w1p.tile([128, S], BF, tag="s4")
                    # wq cb0 + x span0 first (first matmul deps); rest after
                    DMA.dma_start(out=wq_sb[:, 0].rearrange("p a b -> p (a b)"),
                                  in_=wq[:, 0].rearrange("p a b -> p (a b)"))
                    SPB = RS_N // B

                    xtiles = {}

                    def st1_load(rs, fine=False):
                        # split DMAs so the first k-blocks land (and unblock
                        # matmuls) before the whole span arrives; span 0 is
                        # on the critical path so it splits finest
                        xa = xtpa.tile([128, 8, RS_W], BF, tag="xa")
                        xb = xtpb.tile([128, 8, RS_W], BF, tag="xb")
                        xtiles[rs] = (xa, xb)
                        step = 2 if fine else 4
                        for t, src in ((xa, xta[rs]), (xb, xtb[rs])):
                            for o in range(0, 8, step):
                                DMA.dma_start(out=t[:, o:o + step, :],
                                              in_=src[:, o:o + step, :])

                    def st1_span(rs):
                        rsl = slice(rs * RS_W, (rs + 1) * RS_W)
                        ssl = slice((rs % SPB) * RS_W, (rs % SPB + 1) * RS_W)
                        xa, xb = xtiles[rs]
                        if rs == 0:
                            # c4/s4 feed the very first RoPE (~25us in) —
                            # issue them before the span-1 prefetch
                            DMA.dma_start(out=c4_sb[:], in_=c4h[:])
                            DMA.dma_start(out=s4_sb[:], in_=s4h[:])
                            DMA.dma_start(
                                out=wq_sb[:, 1].rearrange("p a b -> p (a b)"),
                                in_=wq[:, 1].rearrange("p a b -> p (a b)"))
                            DMA.dma_start(
                                out=wkv_sb.rearrange("p a b -> p (a b)"),
                                in_=wkv.rearrange("p a b -> p (a b)"))
                        if 1 <= rs <= 4:
                            # trickle in wo (8MB) behind the x stream, fully
                            # loaded by st1 span 4 — op_gen(0) (gated at span
                            # 3) contracts over every wo slot, so late slots
                            # would HOL-block the PE queue
                            wsl = slice(4 * (rs - 1), 4 * rs)
                            DMA.dma_start(
                                out=wo_sb[:, wsl, :].rearrange("p a b -> p (a b)"),
                                in_=wo[:, wsl, :].rearrange("p a b -> p (a b)"))
                        if rs + 1 < RS_N:
                            # prefetch next span's x one full span ahead
                            st1_load(rs + 1)

                        def xt(kb):
                            return xa[:, kb, :] if kb < 8 else xb[:, kb - 8, :]

                        # q projection: 2 colblocks (2 heads each) + RoPE
                        for cb in range(2):
                            pq = ps1.tile([128, RS_W], F32, tag="p1")
                            for kb in range(D // 128):
                                nc.tensor.matmul(
                                    pq[:], wq_sb[:, cb, kb, :],
                                    xt(kb),
                                    start=(kb == 0), stop=(kb == D // 128 - 1))
                                if kb % 4 == 3:
                                    yield
                            # RoPE: qr = pq*C + P2.T @ (pq*S). The rotate-
                            # half matmul overwrites pq IN PLACE (after both
                            # DVE reads) so st1 only ever holds one ps1 tile
                            # — frees a PSUM bank for the op pool.
                            st = ropet.tile([128, RS_W], BF, tag="st")
                            nc.vector.tensor_tensor(out=st[:], in0=pq[:],
                                                    in1=s4_sb[:, ssl],
                                                    op=mybir.AluOpType.mult)
                            ct = ropet.tile([128, RS_W], BF, tag="ct")
                            nc.vector.tensor_tensor(out=ct[:], in0=pq[:],
                                                    in1=c4_sb[:, ssl],
                                                    op=mybir.AluOpType.mult)
                            nc.tensor.matmul(pq[:], p2_sb[:], st[:],
                                             start=True, stop=True)
                            nc.vector.tensor_tensor(out=qrT[cb][:, rsl],
                                                    in0=ct[:], in1=pq[:],
                                                    op=mybir.AluOpType.add)
                            yield

                        # kv projection: cols 0:64 = kT(perm), 64:128 = vT
                        pkv = ps1.tile([128, RS_W], F32, tag="p1")
                        for kb in range(D // 128):
                            nc.tensor.matmul(pkv[:], wkv_sb[:, kb, :], xt(kb),
                                             start=(kb == 0),
                                             stop=(kb == D // 128 - 1))
                            if kb % 4 == 3:
                                yield
                        # k RoPE (partitions 0:64), duplicated into both
                        # krT halves. All pkv reads (stk/ctk/v-bias) are
                        # emitted first, then the rotate-half matmul
                        # overwrites pkv[0:64] in place.
                        stk = ropet.tile([64, RS_W], BF, tag="stk")
                        nc.vector.tensor_tensor(out=stk[:], in0=pkv[0:64, :],
                                                in1=s4_sb[0:64, ssl],
                                                op=mybir.AluOpType.mult)
                        ctk = ropet.tile([64, RS_W], BF, tag="ctk")
                        nc.vector.tensor_tensor(out=ctk[:], in0=pkv[0:64, :],
                                                in1=c4_sb[0:64, ssl],
                                                op=mybir.AluOpType.mult)
                        # v: bias add (vector) + transpose to [key, dim]
                        vst = vstg.tile([64, RS_W], F32, tag="vst")
                        nc.vector.tensor_scalar(out=vst[:], in0=pkv[64:128, :],
                                                scalar1=bv_sb[:], scalar2=None,
                                                op0=mybir.AluOpType.add)
                        nc.tensor.matmul(pkv[0:64, :], p2_sb[0:64, 0:64], stk[:],
                                         start=True, stop=True)
                        nc.vector.tensor_tensor(out=krT[0:64, rsl], in0=ctk[:],
                                                in1=pkv[0:64, :],
                                                op=mybir.AluOpType.add)
                        nc.vector.tensor_tensor(out=krT[64:128, rsl], in0=ctk[:],
                                                in1=pkv[0:64, :],
                                                op=mybir.AluOpType.add)
                        for j in range(RS_W // KB_W):
                            pv = ps1.tile([128, RS_W], F32, tag="p1")
                            nc.tensor.transpose(pv[:, 0:64],
                                                vst[:, j * 128:(j + 1) * 128],
                                                id_sb[:])
                            nc.vector.tensor_copy(
                                out=v_aug[:, rs * (RS_W // KB_W) + j, 0:64],
                                in_=pv[:, 0:64])
                            if j % 2 == 1:
                                yield

                    # ---- out-proj generators (share PSUM tag "p1") ----
                    # operand-swapped: av (128 rows of attnout^T per chunk)
                    # is the stationary operand, wo streams -> N=512 matmuls
                    # (5x fewer PE instructions than the dc-major N=128 form)
                    # and the output lands as y[rows, outdims].
                    ys_t3 = []
                    av3_box = []

                    def op_gen(k):
                        av = avp.tile([128, D // 128, CRW], BF, tag="av")
                        for g in range(2):
                            DMA.dma_start(
                                out=av[:, 8 * g:8 * (g + 1), :],
                                in_=a2a_out[k][g::2].rearrange("b p w -> p b w"))
                        yield
                        for dc4 in range(4):
                            py = ps1.tile([128, RS_W], F32, tag="p1")
                            for kb in range(D // 128):
                                nc.tensor.matmul(
                                    py[:],
                                    av[:, kb, :],
                                    wo_sb[:, kb, dc4 * 512:(dc4 + 1) * 512],
                                    start=(kb == 0), stop=(kb == D // 128 - 1))
                                if kb % 4 == 3 and kb < 15:
                                    yield
                            ys = ystg.tile([128, RS_W], F32, tag="ys")
                            nc.scalar.copy(out=ys[:], in_=py[:])
                            DMA.dma_start(
                                out=y_sh[k * CRW:(k + 1) * CRW,
                                         dc4 * 512:(dc4 + 1) * 512],
                                in_=ys[:])
                            yield

                    def op3_gen_a():
                        # runs entirely post-attention (hides under the cc3b
                        # collective wait). Partial sums for kb 0..7 stay in
                        # PSUM — attention is done, so the ps_s banks are
                        # free; [128,1024] tiles hold two 512-wide dc4
                        # columns each (a single MM may not span banks).
                        av = avp.tile([128, D // 128, CRW], BF, tag="av")
                        av3_box.append(av)
                        DMA.dma_start(out=av[:, 0:8, :],
                                      in_=a2a_out3[0].rearrange("b p w -> p b w"))
                        yield
                        for half in range(2):
                            py = ps_s.tile([128, 2 * QS_W], F32, tag="pss")
                            ys_t3.append(py)
                            for dc4 in (2 * half, 2 * half + 1):
                                col = (dc4 % 2) * 512
                                for kb in range(8):
                                    nc.tensor.matmul(
                                        py[:, col:col + 512],
                                        av[:, kb, :],
                                        wo_sb[:, kb, dc4 * 512:(dc4 + 1) * 512],
                                        start=(kb == 0), stop=False,
                                        skip_group_check=True)
                                    if kb % 4 == 3:
                                        yield

                    def op3_gen_b():
                        # split gather: the first 4 kb blocks unblock the
                        # matmuls ~2us before the full 256KB lands
                        av = av3_box[0]
                        DMA.dma_start(out=av[:, 8:12, :],
                                      in_=a2a_out3[1][0:4].rearrange("b p w -> p b w"))
                        DMA.dma_start(out=av[:, 12:16, :],
                                      in_=a2a_out3[1][4:8].rearrange("b p w -> p b w"))
                        yield
                        for dc4 in range(4):
                            py = ys_t3[dc4 // 2]
                            col = (dc4 % 2) * 512
                            for kb in range(8, D // 128):
                                nc.tensor.matmul(
                                    py[:, col:col + 512],
                                    av[:, kb, :],
                                    wo_sb[:, kb, dc4 * 512:(dc4 + 1) * 512],
                                    start=False, stop=(kb == D // 128 - 1),
                                    skip_group_check=True)
                                if kb % 4 == 3 and kb < 15:
                                    yield
                            # halved staging: DMA of the first half overlaps
                            # the copy of the second, shrinking the drain
                            ys = ystg.tile([128, RS_W], F32, tag="ys")
                            for h in range(2):
                                hc = slice(h * 256, (h + 1) * 256)
                                nc.scalar.copy(out=ys[:, hc],
                                               in_=py[:, col + h * 256:
                                                      col + (h + 1) * 256])
                                DMA.dma_start(
                                    out=y_sh[(NCHK - 1) * CRW:NCHK * CRW,
                                             dc4 * 512 + h * 256:
                                             dc4 * 512 + (h + 1) * 256],
                                    in_=ys[:, hc])
                            yield

                    # ---- interleaved scheduler: stage-1 and out-proj units
                    # are pumped between attention k-blocks so the in-order
                    # PE queue always has work while exp (scalar) runs ----
                    from collections import deque
                    st1q = deque()
                    opq = deque()
                    st1_done = [0]

                    def pump_q(q, is_st1=False):
                        while q:
                            tag, gen = q[0]
                            _mark(tag)
                            try:
                                next(gen)
                                return True
                            except StopIteration:
                                q.popleft()
                                if is_st1:
                                    st1_done[0] += 1
                        return False

                    def drain_st1(r):
                        while st1_done[0] < r and st1q:
                            pump_q(st1q, True)

                    # op units enter only after ALL st1 units are emitted
                    # (they share PSUM banks; interleaving an op accumulation
                    # into an open st1 accumulation could deadlock the PE)
                    GATES = {(3, 0, 0): 0, (5, 1, 4): 1, (6, 1, 0): 2}

                    def pump(s, g, kb):
                        if (s, g, kb) in GATES:
                            kk = GATES[(s, g, kb)]
                            opq.append((f"op{kk}", op_gen(kk)))
                        if st1q:
                            pump_q(st1q, True)
                            pump_q(st1q, True)
                        elif opq:
                            pump_q(opq)
                            if s == 7:
                                pump_q(opq)

                    st1_load(0, fine=True)
                    _mark("st1.0")
                    for _ in st1_span(0):
                        pass
                    for r in range(1, RS_N):
                        st1q.append((f"st1.{r}", st1_span(r)))

                    for s in range(8):
                        b, qs = divmod(s, QS_N)
                        drain_st1(4 * b + qs)
                        if s == 4:
                            drain_st1(7)
                        attn_span(s, pump)
                        if s == 1:
                            emit_cc(0)
                        if s == 3:
                            emit_cc(1)
                        if s == 5:
                            emit_cc(2)
                    drain_st1(7)
                    opq.append(("op3a", op3_gen_a()))
                    opq.append(("op3b", op3_gen_b()))
                    while opq:
                        pump_q(opq)

    _mark("end")
    nc.finalize()
    NAME_ORDER = list(nc._state.inst_map.keys())
    return nc


def _rope_perm():
    return np.concatenate([np.arange(0, HD, 2), np.arange(1, HD, 2)])


def _host_prep(x, Wq, Wk, Wv, bv, Wo, bo):
    """Build per-core input maps (inputs pre-tiled to SBUF layouts)."""
    perm = _rope_perm()

    # x tiled: A[kb, p, r] = x[r, kb*128+p];  xta = kb 0..7, xtb = kb 8..15
    A = np.ascontiguousarray(x.reshape(R, D).T).reshape(D // 128, 128, R)
    xta = np.ascontiguousarray(
        A[0:8].reshape(8, 128, RS_N, RS_W).transpose(2, 1, 0, 3)).astype(BF_NP)
    xtb = np.ascontiguousarray(
        A[8:16].reshape(8, 128, RS_N, RS_W).transpose(2, 1, 0, 3)).astype(BF_NP)

    theta = (1.0 / ROPE_BASE ** (np.arange(0, HD, 2, dtype=np.float64) / HD))
    freqs = np.arange(S, dtype=np.float64)[None, :] * theta[:, None]   # [32, S]
    c4h = np.tile(np.cos(freqs).astype(np.float32), (4, 1)).astype(BF_NP)
    s4h = np.tile(np.sin(freqs).astype(np.float32), (4, 1)).astype(BF_NP)

    p2 = np.zeros((128, 128), dtype=np.float32)
    for p in list(range(0, 32)) + list(range(64, 96)):
        p2[p + 32, p] = -1.0
    for p in list(range(32, 64)) + list(range(96, 128)):
        p2[p - 32, p] = 1.0
    p2 = p2.astype(BF_NP)

    ident = np.eye(64, dtype=np.float32)

    # triangle mask for the 128 diagonal columns: zm[p, w] = (w >= p)
    zm = (np.arange(128)[None, :] >= np.arange(128)[:, None]).astype(
        np.float32).astype(BF_NP)

    # full Wo, shared by every core; row blocks reordered g-major:
    # slot s<8 = (core c=s, t=0), s>=8 = (core c=s-8, t=1)
    wo_r = Wo.reshape(NC, 2, 128, D)
    wo_gm = np.concatenate([wo_r[:, 0], wo_r[:, 1]], axis=0)   # [16,128,D]
    wo_t = np.ascontiguousarray(wo_gm.transpose(1, 0, 2)).astype(BF_NP)

    in_maps = []
    for c in range(NC):
        wq_c = np.empty((D, 256), dtype=np.float32)
        for cb in range(2):
            for u in range(2):
                h = 4 * c + 2 * cb + u
                wq_c[:, cb * 128 + u * 64: cb * 128 + (u + 1) * 64] = Wq[:, h * 64 + perm]
        # cb-major: [128, 2, 16, 128] so each half is one contiguous DMA
        wq_t = np.ascontiguousarray(
            wq_c.reshape(D // 128, 128, 2, 128).transpose(1, 2, 0, 3)).astype(BF_NP)
        wkv_c = np.empty((D, 128), dtype=np.float32)
        wkv_c[:, 0:64] = Wk[:, c * 64 + perm]
        wkv_c[:, 64:128] = Wv[:, c * 64: (c + 1) * 64]
        wkv_t = np.ascontiguousarray(
            wkv_c.reshape(D // 128, 128, 128).transpose(1, 0, 2)).astype(BF_NP)
        bv_c = bv[c * 64:(c + 1) * 64].astype(np.float32).reshape(HD, 1)
        in_maps.append({
            "xta": xta, "xtb": xtb, "wq": wq_t, "wkv": wkv_t, "wo": wo_t,
            "bv": bv_c, "c4h": c4h, "s4h": s4h,
            "p2": p2, "ident": ident, "zm": zm,
        })
    return in_maps


def _run(in_maps, trace=False):
    if "nc" not in _CACHE:
        _CACHE["nc"] = _build()
    try:
        return run_bass_kernel_spmd(_CACHE["nc"], in_maps,
                                    core_ids=list(range(NC)), trace=trace)
    except Exception:
        # transient device wedge happens occasionally; one retry clears it
        return run_bass_kernel_spmd(_CACHE["nc"], in_maps,
                                    core_ids=list(range(NC)), trace=trace)


def _assemble(res, bo):
    Y = np.empty((R, D), dtype=np.float32)
    for j in range(NC):
        yt = np.asarray(res.results[j]["y_sh"], dtype=np.float32)  # [512, D]
        for k in range(NCHK):
            rows = slice(1024 * k + CRW * j, 1024 * k + CRW * (j + 1))
            Y[rows, :] = yt[k * CRW:(k + 1) * CRW, :]
    Y += bo.astype(np.float32)[None, :]
    return Y.reshape(B, S, D)


def kernel(x, Wq, Wk, Wv, bv, Wo, bo, mask):
    """Full inputs -> full output (B, S, D). `mask` is the causal tril mask
    from setup_inputs; causality is hardcoded so it is not shipped to device."""
    in_maps = _host_prep(np.asarray(x), np.asarray(Wq), np.asarray(Wk),
                         np.asarray(Wv), np.asarray(bv), np.asarray(Wo),
                         np.asarray(bo))
    res = _run(in_maps, trace=False)
    return _assemble(res, np.asarray(bo))


def kernel_timed(x, Wq, Wk, Wv, bv, Wo, bo, mask):
    """Like kernel() but with NTFF tracing; returns (y, exec_time_ns)."""
    in_maps = _host_prep(np.asarray(x), np.asarray(Wq), np.asarray(Wk),
                         np.asarray(Wv), np.asarray(bv), np.asarray(Wo),
                         np.asarray(bo))
    res = _run(in_maps, trace=True)
    return _assemble(res, np.asarray(bo)), res.exec_time_ns



# revision 65
# speedup vs baseline: 1.0191x; 1.0191x over previous
"""Trainium2 Bass kernel for causal GQA multi-head attention (nn_MHA_79362405695575).

Full (unsharded) inputs -> full output. Internally: tensor-parallel over heads
across 8 NeuronCores. Core c owns q-heads [4c,4c+4) and kv-head c. After
attention, a small bf16 AllToAll (chunked x4, overlapped with attention)
converts head-sharding to row-sharding; each core then runs the full
out-projection for its own 512 rows of (B*S) and returns y^T for those rows.

Reference semantics (fp32):
  q = x@Wq; k = x@Wk; v = x@Wv + bv           (B=2, S=2048, D=2048)
  q,k := interleaved RoPE(base 10000, hd=64)
  scores = q k^T / 8 (causal), attn = softmax
  out = attn @ v;  y = out @ Wo + bo

All matmul operands are bf16 (PSUM accumulation f32; ~3e-3 rel err, full PE
rate). Everything on-chip is transposed: qT/kT/vT [dim, row] layouts so no PE
transposes are needed anywhere in attention. Softmax is max-free (scores are
provably small) and denominators ride along the AV matmul as a 65th column
of v. Projections (stage 1) are interleaved into the attention stream so the
scalar-engine exp latency of short early spans hides under projection matmuls.
"""

import numpy as np
import ml_dtypes

import concourse.bass as bass
import concourse.tile as tile
from concourse import bacc, mybir
from concourse.bass_utils import run_bass_kernel_spmd

# ---- problem constants (hardcoded; kernel.py must be self-contained) ----
B, S, D = 2, 2048, 2048
NH, NKV, HD = 32, 8, 64
ROPE_BASE = 10000.0
NC = 8                    # cores
HPC = NH // NC            # q heads per core = 4
R = B * S                 # 4096 rows
RS_N = 8                  # projection row spans
RS_W = R // RS_N          # 512 rows per span
QS_W = 512                # attention q-span width
QS_N = 4                  # q spans per batch
KB_W = 128                # k block width
NKB = S // KB_W           # 16 k blocks per batch
NCHK = 4                  # all-to-all chunks (2 spans each)
CRW = R // NCHK // NC     # rows per core per chunk = 128

F32 = mybir.dt.float32
BF = mybir.dt.bfloat16
BF_NP = ml_dtypes.bfloat16

_CACHE = {}

# phase attribution for trace analysis: (tag, #instructions-emitted-so-far)
# pairs + the emission-ordered instruction names. Pure-python bookkeeping —
# the emitted BIR (and thus the NEFF cache key) is unchanged.
PHASE_MARKS = []
NAME_ORDER = None


def _build():
    global NAME_ORDER
    PHASE_MARKS.clear()
    nc = bacc.Bacc("TRN2", target_bir_lowering=False, debug=False, num_devices=NC)

    def _mark(tag):
        PHASE_MARKS.append((tag, len(nc._state.inst_map)))

    # ---- DRAM I/O (pre-tiled on host) ----
    xta = nc.dram_tensor("xta", [RS_N, 128, 8, RS_W], BF, kind="ExternalInput").ap()
    xtb = nc.dram_tensor("xtb", [RS_N, 128, 8, RS_W], BF, kind="ExternalInput").ap()
    wq = nc.dram_tensor("wq", [128, 2, D // 128, 128], BF, kind="ExternalInput").ap()
    wkv = nc.dram_tensor("wkv", [128, D // 128, 128], BF, kind="ExternalInput").ap()
    wo = nc.dram_tensor("wo", [128, D // 128, D], BF, kind="ExternalInput").ap()
    bv_in = nc.dram_tensor("bv", [HD, 1], F32, kind="ExternalInput").ap()
    c4h = nc.dram_tensor("c4h", [128, S], BF, kind="ExternalInput").ap()
    s4h = nc.dram_tensor("s4h", [128, S], BF, kind="ExternalInput").ap()
    p2 = nc.dram_tensor("p2", [128, 128], BF, kind="ExternalInput").ap()
    ident = nc.dram_tensor("ident", [64, 64], F32, kind="ExternalInput").ap()
    zm = nc.dram_tensor("zm", [128, 128], BF, kind="ExternalInput").ap()
    y_sh = nc.dram_tensor("y_sh", [NCHK * CRW, D], F32, kind="ExternalOutput").ap()

    DMA = nc.sync

    with tile.TileContext(nc) as tc:
        with (
            tc.tile_pool(name="persist", bufs=1) as pp,
            tc.tile_pool(name="dram", bufs=1, space="DRAM") as dram,
        ):
            # ---- persistent SBUF (whole kernel) ----
            qrT = [pp.tile([128, R], BF, tag=f"qrT{t}", name=f"qrT{t}") for t in range(2)]
            krT = pp.tile([128, R], BF, tag="krT")
            v_aug = pp.tile([128, R // KB_W, 65], BF, tag="vaug")
            wo_sb = pp.tile([128, D // 128, D], BF, tag="wo")
            p2_sb = pp.tile([128, 128], BF, tag="p2")
            id_sb = pp.tile([64, 64], F32, tag="ident")
            bv_sb = pp.tile([HD, 1], F32, tag="bv")
            zm_sb = pp.tile([128, 128], BF, tag="zm")

            DMA.dma_start(out=p2_sb[:], in_=p2[:])
            DMA.dma_start(out=id_sb[:], in_=ident[:])
            DMA.dma_start(out=bv_sb[:], in_=bv_in[:])
            DMA.dma_start(out=zm_sb[:], in_=zm[:])
            # contiguous whole-tile memset (a strided [:, :, 64:65] memset
            # costs ~14us on the DVE); v copies later overwrite cols 0:64,
            # leaving the ones in col 64
            nc.vector.memset(v_aug[:, :, :], 1.0)

            a2a_in = [dram.tile([16, 128, CRW], BF, tag=f"a2ai{k}", name=f"a2ai{k}")
                      for k in range(NCHK)]
            a2a_out = [dram.tile([16, 128, CRW], BF, tag=f"a2ao{k}", name=f"a2ao{k}")
                       for k in range(NCHK)]
            # last chunk ships per-g so its first collective overlaps the
            # final span's attention (shrinks the end-of-kernel tail)
            a2a_in3 = [dram.tile([8, 128, CRW], BF, tag=f"a2ai3{g}",
                                 name=f"a2ai3{g}") for g in range(2)]
            a2a_out3 = [dram.tile([8, 128, CRW], BF, tag=f"a2ao3{g}",
                                  name=f"a2ao3{g}") for g in range(2)]

            # warmup collective: absorbs the first-collective entry barrier
            # (which eats ALL core-start skew) + firmware setup during stage
            # 1. Tiny payload staged from p2_sb (first DMA, lands ~1us) so
            # the gpsimd queue enters the barrier immediately — the barrier
            # blocks the gpsimd queue (normalize broadcasts + a2a staging),
            # so entering late delays every chunk downstream.
            wu_in = dram.tile([16, 128, 4], BF, tag="wu_i", name="wu_i")
            wu_out = dram.tile([16, 128, 4], BF, tag="wu_o", name="wu_o")
            nc.gpsimd.dma_start(
                out=wu_in.rearrange("b p w -> p b w"),
                in_=p2_sb[:, 0:64].rearrange("p (b w) -> p b w", w=4))
            nc.gpsimd.collective_compute(
                "AllToAll", mybir.AluOpType.bypass,
                replica_groups=[list(range(NC))],
                ins=[wu_in[:]], outs=[wu_out[:]],
            )

            with (
                tc.tile_pool(name="ptp", bufs=3) as ptp,
                tc.tile_pool(name="normp", bufs=2) as normp,
                tc.tile_pool(name="denp", bufs=1) as denp,
                tc.tile_pool(name="sop", bufs=2) as sop,
                tc.tile_pool(name="avp", bufs=2) as avp,
                tc.tile_pool(name="ystg", bufs=5) as ystg,
                tc.tile_pool(name="ps_s", bufs=2, space="PSUM") as ps_s,
                tc.tile_pool(name="ps_av", bufs=1, space="PSUM") as ps_av,
            ):
                def attn_span(s, pump=None):
                    k, sp = divmod(s, 2)
                    b, qs = divmod(s, QS_N)
                    n_kb = 4 * (qs + 1)
                    qsl = slice(b * S + qs * QS_W, b * S + (qs + 1) * QS_W)
                    for g in range(2):
                        pav = ps_av.tile([65, 2 * QS_W], F32, tag="pav")
                        pts = {}

                        def emit_scores(kb):
                            # scores pair (concurrent on disjoint PE row
                            # groups) + exp + diagonal mask for one k-block
                            kbl = slice(b * S + kb * KB_W,
                                        b * S + (kb + 1) * KB_W)
                            off = max(kb - 4 * qs, 0) * 128
                            pss = ps_s.tile([128, 2 * QS_W], F32, tag="pss")
                            for u in range(2):
                                # both u trimmed to off; the exp below reads
                                # the [QS_W, QS_W+off) gap as stale-but-
                                # finite PSUM; its output there is never
                                # consumed
                                usl = slice(u * 64, (u + 1) * 64)
                                nc.tensor.matmul(
                                    pss[:, u * QS_W + off:(u + 1) * QS_W],
                                    krT[usl, kbl],
                                    qrT[g][usl, qsl.start + off:qsl.stop],
                                    start=True, stop=True)
                            pt = ptp.tile([128, 2 * QS_W], BF, tag="pt")
                            pts[kb] = pt
                            nc.scalar.activation(
                                out=pt[:, off:2 * QS_W],
                                in_=pss[:, off:2 * QS_W],
                                func=mybir.ActivationFunctionType.Exp,
                                scale=float(HD) ** -0.5)
                            if kb - 4 * qs >= 0:
                                # triangle mask on the 128 diagonal cols of
                                # each head's valid range
                                for u in range(2):
                                    nc.vector.tensor_tensor(
                                        out=pt[:, u * QS_W + off:
                                            u * QS_W + off + 128],
                                        in0=pt[:, u * QS_W + off:
                                            u * QS_W + off + 128],
                                        in1=zm_sb[:],
                                        op=mybir.AluOpType.mult)

                        # software-pipelined by one k-block: scores(kb+1) and
                        # pump work sit between exp(kb) and AV(kb) in the
                        # in-order PE queue, hiding the ACT exp latency
                        emit_scores(0)
                        for kb in range(n_kb):
                            _mark(f"at{s}g{g}")
                            if kb + 1 < n_kb:
                                emit_scores(kb + 1)
                            if pump is not None:
                                pump(s, g, kb)
                            _mark(f"at{s}g{g}")
                            off = max(kb - 4 * qs, 0) * 128
                            pt = pts.pop(kb)
                            for u in range(2):
                                nc.tensor.matmul(
                                    pav[:, u * QS_W + off:(u + 1) * QS_W],
                                    v_aug[:, b * NKB + kb, :],
                                    pt[:, u * QS_W + off:(u + 1) * QS_W],
                                    start=(kb == 0),
                                    stop=(kb == n_kb - 1),
                                    skip_group_check=True)
                        # normalize heads 2g, 2g+1 and stage for AllToAll.
                        # reciprocal_approx_fast (~18 bits, plenty vs bf16;
                        # ~5x faster than reciprocal()). NOTE: the custom-DVE
                        # op misreads PSUM inputs on hw, so it must read the
                        # SBUF copy, not pav directly.
                        _mark(f"nm{s}g{g}")
                        # normalize: the custom-DVE reciprocal_approx_fast
                        # misreads inputs at non-zero base partition, so first
                        # hop the denominator row (pav row 64) to partition 0
                        # with a plain ACT copy (64->0 remap is 32-aligned,
                        # legal for standard ops), then invert on the DVE.
                        den0 = denp.tile([1, 2 * QS_W], F32, tag="den0")
                        nc.vector.tensor_copy(out=den0[:], in_=pav[64:65, :])
                        den = denp.tile([1, 2 * QS_W], F32, tag="den")
                        nc.vector.reciprocal_approx_fast(
                            out=den[:], in_=den0[:])
                        pavs = normp.tile([65, 2 * QS_W], F32, tag="pavs")
                        nc.scalar.copy(out=pavs[:], in_=pav[:])
                        rb = normp.tile([64, 2 * QS_W], F32, tag="rb")
                        nc.gpsimd.partition_broadcast(rb[:], den[:])
                        so = sop.tile([128, QS_W], BF, tag="so")
                        for u in range(2):
                            nc.vector.tensor_tensor(
                                out=so[u * 64:(u + 1) * 64, :],
                                in0=pavs[0:64, u * QS_W:(u + 1) * QS_W],
                                in1=rb[:, u * QS_W:(u + 1) * QS_W],
                                op=mybir.AluOpType.mult)
                        # scatter: block 2j+g of a2a_in[k] = my rows for core j
                        if k < NCHK - 1:
                            nc.gpsimd.dma_start(
                                out=a2a_in[k][8 * sp + g: 8 * sp + 8: 2]
                                .rearrange("j p w -> p j w"),
                                in_=so.rearrange("p (j w) -> p j w", w=CRW))
                        else:
                            nc.gpsimd.dma_start(
                                out=a2a_in3[g][4 * sp: 4 * sp + 4]
                                .rearrange("j p w -> p j w"),
                                in_=so.rearrange("p (j w) -> p j w", w=CRW))
                            if sp == 1:
                                nc.gpsimd.collective_compute(
                                    "AllToAll", mybir.AluOpType.bypass,
                                    replica_groups=[list(range(NC))],
                                    ins=[a2a_in3[g][:]],
                                    outs=[a2a_out3[g][:]],
                                )

                def emit_cc(k):
                    nc.gpsimd.collective_compute(
                        "AllToAll", mybir.AluOpType.bypass,
                        replica_groups=[list(range(NC))],
                        ins=[a2a_in[k][:]], outs=[a2a_out[k][:]],
                    )

                # ---- stage 1 (projections + RoPE), interleaved below ----
                with (
                    tc.tile_pool(name="w1p", bufs=1) as w1p,
                    tc.tile_pool(name="xtpa", bufs=2) as xtpa,
                    tc.tile_pool(name="xtpb", bufs=2) as xtpb,
                    tc.tile_pool(name="ropet", bufs=2) as ropet,
                    tc.tile_pool(name="vstg", bufs=1) as vstg,
                    tc.tile_pool(name="ps1", bufs=2, space="PSUM") as ps1,
                ):
                    wq_sb = w1p.tile([128, 2, D // 128, 128], BF, tag="wq")
                    wkv_sb = w1p.tile([128, D // 128, 128], BF, tag="wkv")
                    c4_sb = w1p.tile([128, S], BF, tag="c4")
                    s4_sb = w1p.tile([128, S], BF, tag="s4")
                    # wq cb0 + x span0 first (first matmul deps); rest after
                    DMA.dma_start(out=wq_sb[:, 0].rearrange("p a b -> p (a b)"),
                                  in_=wq[:, 0].rearrange("p a b -> p (a b)"))
                    SPB = RS_N // B

                    xtiles = {}

                    def st1_load(rs, fine=False):
                        # split DMAs so the first k-blocks land (and unblock
                        # matmuls) before the whole span arrives; span 0 is
                        # on the critical path so it splits finest
                        xa = xtpa.tile([128, 8, RS_W], BF, tag="xa")
                        xb = xtpb.tile([128, 8, RS_W], BF, tag="xb")
                        xtiles[rs] = (xa, xb)
                        step = 2 if fine else 4
                        for t, src in ((xa, xta[rs]), (xb, xtb[rs])):
                            for o in range(0, 8, step):
                                DMA.dma_start(out=t[:, o:o + step, :],
                                              in_=src[:, o:o + step, :])

                    def st1_span(rs):
                        rsl = slice(rs * RS_W, (rs + 1) * RS_W)
                        ssl = slice((rs % SPB) * RS_W, (rs % SPB + 1) * RS_W)
                        xa, xb = xtiles[rs]
                        if rs == 0:
                            # c4/s4 feed the very first RoPE (~25us in) —
                            # issue them before the span-1 prefetch
                            DMA.dma_start(out=c4_sb[:], in_=c4h[:])
                            DMA.dma_start(out=s4_sb[:], in_=s4h[:])
                            DMA.dma_start(
                                out=wq_sb[:, 1].rearrange("p a b -> p (a b)"),
                                in_=wq[:, 1].rearrange("p a b -> p (a b)"))
                            DMA.dma_start(
                                out=wkv_sb.rearrange("p a b -> p (a b)"),
                                in_=wkv.rearrange("p a b -> p (a b)"))
                        if 1 <= rs <= 4:
                            # trickle in wo (8MB) behind the x stream, fully
                            # loaded by st1 span 4 — op_gen(0) (gated at span
                            # 3) contracts over every wo slot, so late slots
                            # would HOL-block the PE queue
                            wsl = slice(4 * (rs - 1), 4 * rs)
                            DMA.dma_start(
                                out=wo_sb[:, wsl, :].rearrange("p a b -> p (a b)"),
                                in_=wo[:, wsl, :].rearrange("p a b -> p (a b)"))
                        if rs + 1 < RS_N:
                            # prefetch next span's x one full span ahead
                            st1_load(rs + 1)

                        def xt(kb):
                            return xa[:, kb, :] if kb < 8 else xb[:, kb - 8, :]

                        # q projection: 2 colblocks (2 heads each) + RoPE
                        for cb in range(2):
                            pq = ps1.tile([128, RS_W], F32, tag="p1")
                            for kb in range(D // 128):
                                nc.tensor.matmul(
                                    pq[:], wq_sb[:, cb, kb, :],
                                    xt(kb),
                                    start=(kb == 0), stop=(kb == D // 128 - 1))
                                if kb % 4 == 3:
                                    yield
                            # RoPE: qr = pq*C + P2.T @ (pq*S). The rotate-
                            # half matmul overwrites pq IN PLACE (after both
                            # DVE reads) so st1 only ever holds one ps1 tile
                            # — frees a PSUM bank for the op pool.
                            st = ropet.tile([128, RS_W], BF, tag="st")
                            nc.vector.tensor_tensor(out=st[:], in0=pq[:],
                                                    in1=s4_sb[:, ssl],
                                                    op=mybir.AluOpType.mult)
                            ct = ropet.tile([128, RS_W], BF, tag="ct")
                            nc.vector.tensor_tensor(out=ct[:], in0=pq[:],
                                                    in1=c4_sb[:, ssl],
                                                    op=mybir.AluOpType.mult)
                            nc.tensor.matmul(pq[:], p2_sb[:], st[:],
                                             start=True, stop=True)
                            nc.vector.tensor_tensor(out=qrT[cb][:, rsl],
                                                    in0=ct[:], in1=pq[:],
                                                    op=mybir.AluOpType.add)
                            yield

                        # kv projection: cols 0:64 = kT(perm), 64:128 = vT
                        pkv = ps1.tile([128, RS_W], F32, tag="p1")
                        for kb in range(D // 128):
                            nc.tensor.matmul(pkv[:], wkv_sb[:, kb, :], xt(kb),
                                             start=(kb == 0),
                                             stop=(kb == D // 128 - 1))
                            if kb % 4 == 3:
                                yield
                        # k RoPE (partitions 0:64), duplicated into both
                        # krT halves. All pkv reads (stk/ctk/v-bias) are
                        # emitted first, then the rotate-half matmul
                        # overwrites pkv[0:64] in place.
                        stk = ropet.tile([64, RS_W], BF, tag="stk")
                        nc.vector.tensor_tensor(out=stk[:], in0=pkv[0:64, :],
                                                in1=s4_sb[0:64, ssl],
                                                op=mybir.AluOpType.mult)
                        ctk = ropet.tile([64, RS_W], BF, tag="ctk")
                        nc.vector.tensor_tensor(out=ctk[:], in0=pkv[0:64, :],
                                                in1=c4_sb[0:64, ssl],
                                                op=mybir.AluOpType.mult)
                        # v: bias add (vector) + transpose to [key, dim]
                        vst = vstg.tile([64, RS_W], F32, tag="vst")
                        nc.vector.tensor_scalar(out=vst[:], in0=pkv[64:128, :],
                                                scalar1=bv_sb[:], scalar2=None,
                                                op0=mybir.AluOpType.add)
                        nc.tensor.matmul(pkv[0:64, :], p2_sb[0:64, 0:64], stk[:],
                                         start=True, stop=True)
                        nc.vector.tensor_tensor(out=krT[0:64, rsl], in0=ctk[:],
                                                in1=pkv[0:64, :],
                                                op=mybir.AluOpType.add)
                        nc.vector.tensor_tensor(out=krT[64:128, rsl], in0=ctk[:],
                                                in1=pkv[0:64, :],
                                                op=mybir.AluOpType.add)
                        for j in range(RS_W // KB_W):
                            pv = ps1.tile([128, RS_W], F32, tag="p1")
                            nc.tensor.transpose(pv[:, 0:64],
                                                vst[:, j * 128:(j + 1) * 128],
                                                id_sb[:])
                            nc.vector.tensor_copy(
                                out=v_aug[:, rs * (RS_W // KB_W) + j, 0:64],
                                in_=pv[:, 0:64])
                            if j % 2 == 1:
                                yield

                    # ---- out-proj generators (share PSUM tag "p1") ----
                    # operand-swapped: av (128 rows of attnout^T per chunk)
                    # is the stationary operand, wo streams -> N=512 matmuls
                    # (5x fewer PE instructions than the dc-major N=128 form)
                    # and the output lands as y[rows, outdims].
                    ys_t3 = []
                    av3_box = []

                    def op_gen(k):
                        av = avp.tile([128, D // 128, CRW], BF, tag="av")
                        for g in range(2):
                            DMA.dma_start(
                                out=av[:, 8 * g:8 * (g + 1), :],
                                in_=a2a_out[k][g::2].rearrange("b p w -> p b w"))
                        yield
                        for dc4 in range(4):
                            py = ps1.tile([128, RS_W], F32, tag="p1")
                            for kb in range(D // 128):
                                nc.tensor.matmul(
                                    py[:],
                                    av[:, kb, :],
                                    wo_sb[:, kb, dc4 * 512:(dc4 + 1) * 512],
                                    start=(kb == 0), stop=(kb == D // 128 - 1))
                                if kb % 4 == 3 and kb < 15:
                                    yield
                            ys = ystg.tile([128, RS_W], F32, tag="ys")
                            nc.scalar.copy(out=ys[:], in_=py[:])
                            DMA.dma_start(
                                out=y_sh[k * CRW:(k + 1) * CRW,
                                         dc4 * 512:(dc4 + 1) * 512],
                                in_=ys[:])
                            yield

                    def op3_gen_a():
                        # runs entirely post-attention (hides under the cc3b
                        # collective wait). Partial sums for kb 0..7 stay in
                        # PSUM — attention is done, so the ps_s banks are
                        # free; [128,1024] tiles hold two 512-wide dc4
                        # columns each (a single MM may not span banks).
                        av = avp.tile([128, D // 128, CRW], BF, tag="av")
                        av3_box.append(av)
                        DMA.dma_start(out=av[:, 0:8, :],
                                      in_=a2a_out3[0].rearrange("b p w -> p b w"))
                        yield
                        for half in range(2):
                            py = ps_s.tile([128, 2 * QS_W], F32, tag="pss")
                            ys_t3.append(py)
                            for dc4 in (2 * half, 2 * half + 1):
                                col = (dc4 % 2) * 512
                                for kb in range(8):
                                    nc.tensor.matmul(
                                        py[:, col:col + 512],
                                        av[:, kb, :],
                                        wo_sb[:, kb, dc4 * 512:(dc4 + 1) * 512],
                                        start=(kb == 0), stop=False,
                                        skip_group_check=True)
                                    if kb % 4 == 3:
                                        yield

                    def op3_gen_b():
                        av = av3_box[0]
                        DMA.dma_start(out=av[:, 8:16, :],
                                      in_=a2a_out3[1].rearrange("b p w -> p b w"))
                        yield
                        for dc4 in range(4):
                            py = ys_t3[dc4 // 2]
                            col = (dc4 % 2) * 512
                            for kb in range(8, D // 128):
                                nc.tensor.matmul(
                                    py[:, col:col + 512],
                                    av[:, kb, :],
                                    wo_sb[:, kb, dc4 * 512:(dc4 + 1) * 512],
                                    start=False, stop=(kb == D // 128 - 1),
                                    skip_group_check=True)
                                if kb % 4 == 3 and kb < 15:
                                    yield
                            ys = ystg.tile([128, RS_W], F32, tag="ys")
                            nc.scalar.copy(out=ys[:], in_=py[:, col:col + 512])
                            DMA.dma_start(
                                out=y_sh[(NCHK - 1) * CRW:NCHK * CRW,
                                         dc4 * 512:(dc4 + 1) * 512],
                                in_=ys[:])
                            yield

                    # ---- interleaved scheduler: stage-1 and out-proj units
                    # are pumped between attention k-blocks so the in-order
                    # PE queue always has work while exp (scalar) runs ----
                    from collections import deque
                    st1q = deque()
                    opq = deque()
                    st1_done = [0]

                    def pump_q(q, is_st1=False):
                        while q:
                            tag, gen = q[0]
                            _mark(tag)
                            try:
                                next(gen)
                                return True
                            except StopIteration:
                                q.popleft()
                                if is_st1:
                                    st1_done[0] += 1
                        return False

                    def drain_st1(r):
                        while st1_done[0] < r and st1q:
                            pump_q(st1q, True)

                    # op units enter only after ALL st1 units are emitted
                    # (they share PSUM banks; interleaving an op accumulation
                    # into an open st1 accumulation could deadlock the PE)
                    GATES = {(3, 0, 0): 0, (5, 1, 0): 1, (6, 1, 0): 2}

                    def pump(s, g, kb):
                        if (s, g, kb) in GATES:
                            kk = GATES[(s, g, kb)]
                            opq.append((f"op{kk}", op_gen(kk)))
                        if st1q:
                            pump_q(st1q, True)
                            pump_q(st1q, True)
                        elif opq:
                            pump_q(opq)
                            if s == 7:
                                pump_q(opq)

                    st1_load(0, fine=True)
                    _mark("st1.0")
                    for _ in st1_span(0):
                        pass
                    for r in range(1, RS_N):
                        st1q.append((f"st1.{r}", st1_span(r)))

                    for s in range(8):
                        b, qs = divmod(s, QS_N)
                        drain_st1(4 * b + qs)
                        if s == 4:
                            drain_st1(7)
                        attn_span(s, pump)
                        if s == 1:
                            emit_cc(0)
                        if s == 3:
                            emit_cc(1)
                        if s == 5:
                            emit_cc(2)
                    drain_st1(7)
                    opq.append(("op3a", op3_gen_a()))
                    opq.append(("op3b", op3_gen_b()))
                    while opq:
                        pump_q(opq)

    _mark("end")
    nc.finalize()
    NAME_ORDER = list(nc._state.inst_map.keys())
    return nc


def _rope_perm():
    return np.concatenate([np.arange(0, HD, 2), np.arange(1, HD, 2)])


def _host_prep(x, Wq, Wk, Wv, bv, Wo, bo):
    """Build per-core input maps (inputs pre-tiled to SBUF layouts)."""
    perm = _rope_perm()

    # x tiled: A[kb, p, r] = x[r, kb*128+p];  xta = kb 0..7, xtb = kb 8..15
    A = np.ascontiguousarray(x.reshape(R, D).T).reshape(D // 128, 128, R)
    xta = np.ascontiguousarray(
        A[0:8].reshape(8, 128, RS_N, RS_W).transpose(2, 1, 0, 3)).astype(BF_NP)
    xtb = np.ascontiguousarray(
        A[8:16].reshape(8, 128, RS_N, RS_W).transpose(2, 1, 0, 3)).astype(BF_NP)

    theta = (1.0 / ROPE_BASE ** (np.arange(0, HD, 2, dtype=np.float64) / HD))
    freqs = np.arange(S, dtype=np.float64)[None, :] * theta[:, None]   # [32, S]
    c4h = np.tile(np.cos(freqs).astype(np.float32), (4, 1)).astype(BF_NP)
    s4h = np.tile(np.sin(freqs).astype(np.float32), (4, 1)).astype(BF_NP)

    p2 = np.zeros((128, 128), dtype=np.float32)
    for p in list(range(0, 32)) + list(range(64, 96)):
        p2[p + 32, p] = -1.0
    for p in list(range(32, 64)) + list(range(96, 128)):
        p2[p - 32, p] = 1.0
    p2 = p2.astype(BF_NP)

    ident = np.eye(64, dtype=np.float32)

    # triangle mask for the 128 diagonal columns: zm[p, w] = (w >= p)
    zm = (np.arange(128)[None, :] >= np.arange(128)[:, None]).astype(
        np.float32).astype(BF_NP)

    # full Wo, shared by every core; row blocks reordered g-major:
    # slot s<8 = (core c=s, t=0), s>=8 = (core c=s-8, t=1)
    wo_r = Wo.reshape(NC, 2, 128, D)
    wo_gm = np.concatenate([wo_r[:, 0], wo_r[:, 1]], axis=0)   # [16,128,D]
    wo_t = np.ascontiguousarray(wo_gm.transpose(1, 0, 2)).astype(BF_NP)

    in_maps = []
    for c in range(NC):
        wq_c = np.empty((D, 256), dtype=np.float32)
        for cb in range(2):
            for u in range(2):
                h = 4 * c + 2 * cb + u
                wq_c[:, cb * 128 + u * 64: cb * 128 + (u + 1) * 64] = Wq[:, h * 64 + perm]
        # cb-major: [128, 2, 16, 128] so each half is one contiguous DMA
        wq_t = np.ascontiguousarray(
            wq_c.reshape(D // 128, 128, 2, 128).transpose(1, 2, 0, 3)).astype(BF_NP)
        wkv_c = np.empty((D, 128), dtype=np.float32)
        wkv_c[:, 0:64] = Wk[:, c * 64 + perm]
        wkv_c[:, 64:128] = Wv[:, c * 64: (c + 1) * 64]
        wkv_t = np.ascontiguousarray(
            wkv_c.reshape(D // 128, 128, 128).transpose(1, 0, 2)).astype(BF_NP)
        bv_c = bv[c * 64:(c + 1) * 64].astype(np.float32).reshape(HD, 1)
        in_maps.append({
            "xta": xta, "xtb": xtb, "wq": wq_t, "wkv": wkv_t, "wo": wo_t,
            "bv": bv_c, "c4h": c4h, "s4h": s4h,
            "p2": p2, "ident": ident, "zm": zm,
        })
    return in_maps


def _run(in_maps, trace=False):
    if "nc" not in _CACHE:
        _CACHE["nc"] = _build()
    try:
        return run_bass_kernel_spmd(_CACHE["nc"], in_maps,
                                    core_ids=list(range(NC)), trace=trace)
    except Exception:
        # transient device wedge happens occasionally; one retry clears it
        return run_bass_kernel_spmd(_CACHE["nc"], in_maps,
                                    core_ids=list(range(NC)), trace=trace)


def _assemble(res, bo):
    Y = np.empty((R, D), dtype=np.float32)
    for j in range(NC):
        yt = np.asarray(res.results[j]["y_sh"], dtype=np.float32)  # [512, D]
        for k in range(NCHK):
            rows = slice(1024 * k + CRW * j, 1024 * k + CRW * (j + 1))
            Y[rows, :] = yt[k * CRW:(k + 1) * CRW, :]
    Y += bo.astype(np.float32)[None, :]
    return Y.reshape(B, S, D)


def kernel(x, Wq, Wk, Wv, bv, Wo, bo, mask):
    """Full inputs -> full output (B, S, D). `mask` is the causal tril mask
    from setup_inputs; causality is hardcoded so it is not shipped to device."""
    in_maps = _host_prep(np.asarray(x), np.asarray(Wq), np.asarray(Wk),
                         np.asarray(Wv), np.asarray(bv), np.asarray(Wo),
                         np.asarray(bo))
    res = _run(in_maps, trace=False)
    return _assemble(res, np.asarray(bo))


def kernel_timed(x, Wq, Wk, Wv, bv, Wo, bo, mask):
    """Like kernel() but with NTFF tracing; returns (y, exec_time_ns)."""
    in_maps = _host_prep(np.asarray(x), np.asarray(Wq), np.asarray(Wk),
                         np.asarray(Wv), np.asarray(bv), np.asarray(Wo),
                         np.asarray(bo))
    res = _run(in_maps, trace=True)
    return _assemble(res, np.asarray(bo)), res.exec_time_ns

